# revision 16
# baseline (speedup 1.0000x reference)
"""Trainium2 Bass kernel for nn_LGnet (LSTM + memory attention recurrence).

Sharding: data-parallel over batch, B=256 -> 32 rows per core across 8 cores.
All on-chip state is kept transposed ([feature partitions, batch free]).

Algebraic restructure: logits = memory @ (ls_z[t] + WQ3F @ h)
                              = lgz[t] + WM @ h
where lgz = memory @ ls_z is precomputed on-chip (fp16, off the critical
path) and WM = 0.5 * memory @ WQ3F is a host-precomputed fp16 constant
(consuming h2 = 2h). This removes the per-step ls matmul + PSUM->SBUF
round-trip from the serial chain entirely: the logits matmuls depend only
on h and constants, so they issue the moment h is ready.

Step structure (per t):
  logits = lgz[t] + WM @ h2      (4x[ident + 4 wm] fp16 matmuls)
  e = exp(logits); s = ones-matmul sum; r = 1/s (bcast via ones matmul)
  gd = (e @ memory) * r          (bf16 matmuls)
  gates = bias + W_hh' @ h2 + W_ih' @ gdn  (fp16; bias via one K=16
          selector matmul - single closed PSUM group, rest is RMW;
          i/f/o rows pre-scaled 0.5, W_hh cols 0.5)
  Y = tanh(gates)                (ONE [128,512] activation)
  u = (Yf+1)*ch; v = (Yi+1)*Yg; cn2 = 2u+v  (fused scalar_tensor_tensor)
  TC = tanh(0.5*cn2); hb = (Yo+1)*TC -> fp16 (=2h); ch = 0.25*cn2 (=c/2)
The h-dependent gate matmuls fill the softmax window on the tensor engine.
"""
import os
import numpy as np
from contextlib import ExitStack

B, T, F, H, O, M = 256, 100, 128, 512, 128, 512
T = int(os.environ.get("LG_T", str(T)))   # debug override; harness uses 100
NC = 8
BB = B // NC          # 32 batch rows per core
TB = T * BB           # 3200 columns in (t, b) packing
NTCH = 4              # precompute T-chunks
TCH = T // NTCH       # 25 steps per chunk
CCH = TCH * BB        # 800 cols per chunk

_built = None


def _build():
    import concourse.bass as bass
    import concourse.tile as tile
    from concourse import bacc, mybir, bass_isa

    f32 = mybir.dt.float32
    bf16 = mybir.dt.bfloat16
    f16 = mybir.dt.float16
    AF = mybir.ActivationFunctionType
    ALU = mybir.AluOpType
    nc = bacc.Bacc("TRN2", target_bir_lowering=False, debug=False, num_devices=NC)
    # ---- DRAM tensors (per-core data fed via in_maps) ----
    dt_in = {}
    for name in ["x", "xl", "mask", "delta", "xlb", "dltb", "xmb"]:
        dt_in[name] = nc.dram_tensor(name, [F, TB], f32, kind="ExternalInput").ap()
    wg_d = nc.dram_tensor("wg", [128, 80 * 128], f16, kind="ExternalInput").ap()
    selm_d = nc.dram_tensor("selm", [16, 512], f16, kind="ExternalInput").ap()
    biast16_d = nc.dram_tensor("biast16", [16, 128], f16, kind="ExternalInput").ap()
    wm_d = nc.dram_tensor("wm", [128, 16 * 128], f16, kind="ExternalInput").ap()
    ident_d = nc.dram_tensor("ident", [128, 128], f16, kind="ExternalInput").ap()
    memt_d = nc.dram_tensor("memt", [128, 512], f16, kind="ExternalInput").ap()
    membf_d = nc.dram_tensor("membf", [128, 512], bf16, kind="ExternalInput").ap()
    wfct_d = nc.dram_tensor("wfct", [128, 512], f32, kind="ExternalInput").ap()
    wqz_d = nc.dram_tensor("wqz", [128, 128], f32, kind="ExternalInput").ap()
    wqzp_d = nc.dram_tensor("wqzp", [128, 128], f32, kind="ExternalInput").ap()
    scal_d = nc.dram_tensor("scal", [128, 8], f32, kind="ExternalInput").ap()
    # scal cols: 0 dgz, 1 bgz, 2 dgzp, 3 bgzp, 4 b_q_eff, 5 b_fc
    o_d = nc.dram_tensor("o", [O, BB], f32, kind="ExternalOutput").ap()
    dbg = os.environ.get("LG_DEBUG") == "1"
    if dbg:
        dbg_d = {nm: nc.dram_tensor(f"dbg_{nm}", shp, f32, kind="ExternalOutput").ap()
                 for nm, shp in [("eT", [128, 128]),
                                 ("gdn", [128, BB]), ("G", [128, 512]),
                                 ("Y", [128, 512]), ("cn2", [128, 128]),
                                 ("hb", [128, 128])]}

    with tile.TileContext(nc) as tc, ExitStack() as ctx:
        wpool = ctx.enter_context(tc.tile_pool(name="wpool", bufs=1))
        inp = ctx.enter_context(tc.tile_pool(name="inp", bufs=2))
        pre = ctx.enter_context(tc.tile_pool(name="pre", bufs=2))
        lszp = ctx.enter_context(tc.tile_pool(name="lszp", bufs=1))
        stp = ctx.enter_context(tc.tile_pool(name="stp", bufs=2))
        state = ctx.enter_context(tc.tile_pool(name="state", bufs=2))
        pers = ctx.enter_context(tc.tile_pool(name="pers", bufs=1))
        attn_ps = ctx.enter_context(tc.tile_pool(name="attn_ps", bufs=2, space="PSUM"))
        gates_ps = ctx.enter_context(tc.tile_pool(name="gates_ps", bufs=2, space="PSUM"))
        pre_ps = ctx.enter_context(tc.tile_pool(name="pre_ps", bufs=2, space="PSUM"))

        # ---- static weights into SBUF ----
        WG = wpool.tile([128, 80 * 128], f16, tag="WG")
        nc.sync.dma_start(WG[:], wg_d[:])
        SELM = wpool.tile([16, 512], f16, tag="SELM")
        nc.sync.dma_start(SELM[:], selm_d[:])
        BIAST16 = wpool.tile([16, 128], f16, tag="BIAST16")
        nc.sync.dma_start(BIAST16[:], biast16_d[:])
        WM = wpool.tile([128, 16 * 128], f16, tag="WM")
        nc.sync.dma_start(WM[:], wm_d[:])
        IDENT = wpool.tile([128, 128], f16, tag="IDENT")
        nc.sync.dma_start(IDENT[:], ident_d[:])
        MEMT = wpool.tile([128, 512], f16, tag="MEMT")
        nc.sync.dma_start(MEMT[:], memt_d[:])
        MEMBF = wpool.tile([128, 512], bf16, tag="MEMBF")
        nc.sync.dma_start(MEMBF[:], membf_d[:])
        WFCT = wpool.tile([128, 512], f32, tag="WFCT")
        nc.sync.dma_start(WFCT[:], wfct_d[:])
        WQZ = wpool.tile([128, 128], f32, tag="WQZ")
        nc.sync.dma_start(WQZ[:], wqz_d[:])
        WQZP = wpool.tile([128, 128], f32, tag="WQZP")
        nc.sync.dma_start(WQZP[:], wqzp_d[:])
        SCAL = wpool.tile([128, 8], f32, tag="SCAL")
        nc.sync.dma_start(SCAL[:], scal_d[:])
        ONESF = wpool.tile([128, 128], bf16, tag="ONESF")
        nc.vector.memset(ONESF[:], 1.0)
        ONESC = wpool.tile([128, 1], bf16, tag="ONESC")
        nc.vector.memset(ONESC[:], 1.0)

        dgz, bgz = SCAL[:, 0:1], SCAL[:, 1:2]
        dgzp, bgzp = SCAL[:, 2:3], SCAL[:, 3:4]
        bq_ap, bfc_ap = SCAL[:, 4:5], SCAL[:, 5:6]

        # ---- persistent tiles ----
        ls_z = lszp.tile([128, TB], f16, tag="ls_z")
        lgz = lszp.tile([128, 4 * TB], f16, tag="lgz")   # memory @ ls_z, 4 m-chunks

        hb = pers.tile([128, 128], f16, tag="hb")      # 2h, fp16
        ch = pers.tile([128, 128], f32, tag="ch")      # c/2, fp32
        nc.vector.memset(hb[:], 0.0)
        nc.vector.memset(ch[:], 0.0)

        # ---- precompute z/zp, ls_z and lgz = memory @ ls_z in T-chunks ----
        with nc.named_scope("precompute"):
            for cc in range(NTCH):
                sl = slice(cc * CCH, (cc + 1) * CCH)
                chd = {}
                for name in ["x", "xl", "mask", "delta", "xlb", "dltb", "xmb"]:
                    t_ = inp.tile([128, CCH], f32, tag=f"in_{name}")
                    nc.sync.dma_start(t_[:], dt_in[name][:, sl])
                    chd[name] = t_

                def zchain(dsrc, xlsrc, dg, bg, tag):
                    r1 = pre.tile([128, CCH], f32, tag="tA")
                    nc.scalar.activation(r1[:], dsrc[:], AF.Relu, scale=dg, bias=bg)
                    dz = pre.tile([128, CCH], f32, tag="tB")
                    nc.scalar.activation(dz[:], r1[:], AF.Exp, scale=-1.0)
                    u = pre.tile([128, CCH], f32, tag="tA")
                    nc.vector.tensor_tensor(u[:], xlsrc[:], chd["xmb"][:], ALU.subtract)
                    v = pre.tile([128, CCH], f32, tag="tB2")
                    nc.vector.tensor_tensor(v[:], dz[:], u[:], ALU.mult)
                    w = pre.tile([128, CCH], f32, tag="tC")
                    nc.vector.tensor_tensor(w[:], v[:], chd["xmb"][:], ALU.add)
                    d_ = pre.tile([128, CCH], f32, tag="tA")
                    nc.vector.tensor_tensor(d_[:], chd["x"][:], w[:], ALU.subtract)
                    e2 = pre.tile([128, CCH], f32, tag="tB")
                    nc.vector.tensor_tensor(e2[:], chd["mask"][:], d_[:], ALU.mult)
                    z_ = pre.tile([128, CCH], f32, tag=f"z{tag}")
                    nc.vector.tensor_tensor(z_[:], w[:], e2[:], ALU.add)
                    return z_

                z_c = zchain(chd["delta"], chd["xl"], dgz, bgz, "z")
                zp_c = zchain(chd["dltb"], chd["xlb"], dgzp, bgzp, "p")

                for off in range(0, CCH, 512):
                    n = min(512, CCH - off)
                    pp = pre_ps.tile([128, 512], f32, tag="pp")
                    nc.tensor.matmul(pp[:, :n], lhsT=WQZ[:], rhs=z_c[:, off:off + n],
                                     start=True, stop=False)
                    nc.tensor.matmul(pp[:, :n], lhsT=WQZP[:], rhs=zp_c[:, off:off + n],
                                     start=False, stop=True)
                    nc.scalar.activation(ls_z[:, cc * CCH + off: cc * CCH + off + n],
                                         pp[:, :n], AF.Identity, bias=bq_ap)
                # lgz chunks for this cc: 4 m-chunks x (512+288) cols
                for j in range(4):
                    for off in range(0, CCH, 512):
                        n = min(512, CCH - off)
                        pq = pre_ps.tile([128, 512], f32, tag="pq")
                        nc.tensor.matmul(pq[:, :n], lhsT=MEMT[:, 128 * j:128 * (j + 1)],
                                         rhs=ls_z[:, cc * CCH + off: cc * CCH + off + n],
                                         start=True, stop=True)
                        dst = lgz[:, j * TB + cc * CCH + off: j * TB + cc * CCH + off + n]
                        nc.scalar.activation(dst, pq[:, :n], AF.Identity)

        # gate-group matmul helpers ------------------------------------
        def g_h(pg, g):
            # 4 h-chunk matmuls accumulating (RMW) onto the bias written by
            # the single selector matmul
            for k in range(4):
                nc.tensor.matmul(pg[:, 32 * g:32 * g + 32],
                                 lhsT=WG[:, 128 * (g * 5 + 1 + k):128 * (g * 5 + 2 + k)],
                                 rhs=hb[:, 32 * k:32 * k + 32],
                                 start=False, stop=False, skip_group_check=True)

        def g_gd(pg, g, gdn):
            nc.tensor.matmul(pg[:, 32 * g:32 * g + 32],
                             lhsT=WG[:, 128 * (g * 5):128 * (g * 5 + 1)],
                             rhs=gdn[:], start=False, stop=True, skip_group_check=True)

        # ---- recurrence ----
        for t in range(T):
            with nc.named_scope(f"step{t}" if t % 10 == 0 else "step"):
                pa = attn_ps.tile([128, 512], f32, tag="pa")
                pg = gates_ps.tile([128, 512], f32, tag="pg")
                # logits^T chunk j = lgz[t] chunk j + sum_k WM(j,k) @ h2 chunk k
                for j in range(4):
                    rg = pa[:, 128 + 32 * j:128 + 32 * (j + 1)]
                    nc.tensor.matmul(rg, lhsT=IDENT[:],
                                     rhs=lgz[:, j * TB + 32 * t: j * TB + 32 * t + 32],
                                     start=True, stop=False)
                    for k in range(4):
                        nc.tensor.matmul(rg, lhsT=WM[:, 128 * (4 * j + k):128 * (4 * j + k + 1)],
                                         rhs=hb[:, 32 * k:32 * k + 32],
                                         start=False, stop=(k == 3))
                # bias for all 16 groups via one K=16 selector matmul (single
                # closed accumulation group; everything after is RMW)
                nc.tensor.matmul(pg[:, 0:512], lhsT=BIAST16[:], rhs=SELM[:],
                                 start=True, stop=True)
                eT = stp.tile([128, 128], bf16, tag="eT")
                nc.scalar.activation(eT[:], pa[:, 128:256], AF.Exp)
                # fold the 4 m-chunks on vector, then all-reduce the [128,32]
                # partials over partitions on gpsimd (result broadcast to all
                # partitions -> no ones-matmul, no broadcast matmul)
                eF = stp.tile([128, BB], f32, tag="eF")
                nc.vector.tensor_reduce(eF[:], eT[:].rearrange("p (c b) -> p b c", c=4),
                                        axis=mybir.AxisListType.X, op=ALU.add)
                eAR = stp.tile([128, BB], f32, tag="eAR")
                nc.gpsimd.partition_all_reduce(eAR[:], eF[:], channels=128,
                                               reduce_op=bass_isa.ReduceOp.add)
                # gd^T = memory.T-chunks @ e^T  (bf16)
                for j in range(4):
                    nc.tensor.matmul(pa[:, 256:288], lhsT=MEMBF[:, 128 * j:128 * (j + 1)],
                                     rhs=eT[:, 32 * j:32 * j + 32],
                                     start=(j == 0), stop=(j == 3))
                # gates: h part groups 0..9 (fills the softmax window)
                for g in range(0, 10):
                    g_h(pg, g)
                R = stp.tile([128, BB], f32, tag="R")
                nc.vector.reciprocal(R[:], eAR[:])
                for g in range(10, 16):
                    g_h(pg, g)
                gdn = stp.tile([128, BB], f16, tag="gdn")
                nc.vector.tensor_tensor(gdn[:], pa[:, 256:288], R[:], ALU.mult)
                # gd matmuls and tanh interleaved per gate block (f, then ig,
                # then o) so each activation starts as soon as its block is done
                Y = stp.tile([128, 512], f32, tag="Y")
                for g in range(0, 4):
                    g_gd(pg, g, gdn)
                nc.scalar.activation(Y[:, 0:128], pg[:, 0:128], AF.Tanh)
                for g in range(4, 12):
                    g_gd(pg, g, gdn)
                nc.scalar.activation(Y[:, 128:384], pg[:, 128:384], AF.Tanh)
                for g in range(12, 16):
                    g_gd(pg, g, gdn)
                nc.scalar.activation(Y[:, 384:512], pg[:, 384:512], AF.Tanh)
                # pointwise (group order f,i,g,o): u=(Yf+1)*ch  v=(Yi+1)*Yg
                u = stp.tile([128, 128], f32, tag="u")
                nc.vector.scalar_tensor_tensor(u[:], Y[:, 0:128], 1.0, ch[:],
                                               ALU.add, ALU.mult)
                v = stp.tile([128, 128], f32, tag="v")
                nc.vector.scalar_tensor_tensor(v[:], Y[:, 128:256], 1.0, Y[:, 256:384],
                                               ALU.add, ALU.mult)
                cn2 = state.tile([128, 128], f32, tag="cn2")
                nc.vector.scalar_tensor_tensor(cn2[:], v[:], 0.5, u[:],
                                               ALU.mult, ALU.add)
                TC = stp.tile([128, 128], f32, tag="TC")
                nc.scalar.activation(TC[:], cn2[:], AF.Tanh)
                hb_new = state.tile([128, 128], f16, tag="hb2")
                nc.vector.scalar_tensor_tensor(hb_new[:], Y[:, 384:512], 1.0, TC[:],
                                               ALU.add, ALU.mult)
                ch_new = state.tile([128, 128], f32, tag="ch2")
                nc.scalar.mul(ch_new[:], cn2[:], 0.5)
                if dbg and t == int(os.environ.get("LG_DBGT", "0")):
                    eTf = stp.tile([128, 128], f32, tag="eTf")
                    nc.vector.tensor_copy(eTf[:], eT[:])
                    nc.sync.dma_start(dbg_d["eT"][:], eTf[:])
                    gdnf = stp.tile([128, BB], f32, tag="gdnf")
                    nc.vector.tensor_copy(gdnf[:], gdn[:])
                    nc.sync.dma_start(dbg_d["gdn"][:], gdnf[:])
                    Gd = stp.tile([128, 512], f32, tag="Gd")
                    nc.scalar.activation(Gd[:], pg[:], AF.Identity)
                    nc.sync.dma_start(dbg_d["G"][:], Gd[:])
                    nc.sync.dma_start(dbg_d["Y"][:], Y[:])
                    nc.sync.dma_start(dbg_d["cn2"][:], cn2[:])
                    hbf = stp.tile([128, 128], f32, tag="hbf")
                    nc.vector.tensor_copy(hbf[:], hb_new[:])
                    nc.sync.dma_start(dbg_d["hb"][:], hbf[:])
                if t == T - 1:
                    h2f = stp.tile([128, 128], f32, tag="h2f")
                    nc.vector.scalar_tensor_tensor(h2f[:], Y[:, 384:512], 1.0, TC[:],
                                                   ALU.add, ALU.mult)
                hb, ch = hb_new, ch_new

        # ---- final output: out^T = 0.5*W_fc @ h2 + b_fc ----
        with nc.named_scope("final"):
            pf = attn_ps.tile([128, 512], f32, tag="pa")
            for k in range(4):
                nc.tensor.matmul(pf[:, 0:32], lhsT=WFCT[:, 128 * k:128 * (k + 1)],
                                 rhs=h2f[:, 32 * k:32 * k + 32],
                                 start=(k == 0), stop=(k == 3))
            outt = stp.tile([O, BB], f32, tag="outt")
            nc.scalar.activation(outt[:], pf[:, 0:32], AF.Identity, bias=bfc_ap)
            nc.sync.dma_start(o_d[:], outt[:])

    nc.compile()
    return nc


def _prep_host(inputs):
    """Host-side: fold weights, build per-core input maps."""
    inp = {k: np.asarray(v, np.float32) for k, v in inputs.items()}
    dgz = np.ascontiguousarray(np.diag(inp["W_gz"]))
    dgzp = np.ascontiguousarray(np.diag(inp["W_gzp"]))
    Wq = inp["W_q"]
    WQ3F = (Wq[:, 2 * F:] @ inp["W_fc"]).astype(np.float32)       # [F, H]
    b_q_eff = (inp["b_q"] + Wq[:, 2 * F:] @ inp["b_fc"]).astype(np.float32)
    bias_g = (inp["b_ih"] + inp["b_hh"]).astype(np.float32)       # [2048]

    # gates weights: Wcat.T tiles; tile (g, k): k=0 -> W_ih cols, k=1..4 -> W_hh
    # pre-scale: i/f/o gate columns (g//4 != 2) x0.5 for the tanh-sigmoid trick,
    # W_hh part (k>=1) x0.5 because h state is stored as 2h.
    WcatT = np.concatenate([inp["W_ih"], inp["W_hh"]], axis=1).T  # [640, 2048]
    # group order in pg columns: [f(4..7), i(0..3), g(8..11), o(12..15)] so a
    # small tanh over the f block can unblock the c-update early
    GORD = [4, 5, 6, 7, 0, 1, 2, 3, 8, 9, 10, 11, 12, 13, 14, 15]
    wg = np.empty((128, 80 * 128), np.float32)
    for gn, g in enumerate(GORD):
        gate_sc = 0.5 if g // 4 != 2 else 1.0
        for k in range(5):
            sc = gate_sc * (0.5 if k >= 1 else 1.0)
            wg[:, 128 * (gn * 5 + k):128 * (gn * 5 + k + 1)] = \
                sc * WcatT[128 * k:128 * (k + 1), 128 * g:128 * (g + 1)]

    selm = np.zeros((16, 512), np.float32)
    for g in range(16):
        selm[g, 32 * g:32 * (g + 1)] = 1.0
    biast16 = np.empty((16, 128), np.float32)
    for gn, g in enumerate(GORD):
        gate_sc = 0.5 if g // 4 != 2 else 1.0
        biast16[gn, :] = gate_sc * bias_g[128 * g:128 * (g + 1)]

    # WM = 0.5 * memory @ WQ3F  [M, H]; tile (j, k) = WM[mj, hk].T
    WMh = 0.5 * (inp["memory"] @ WQ3F)                            # [512, 512]
    wm = np.empty((128, 16 * 128), np.float32)
    for j in range(4):
        for k in range(4):
            wm[:, 128 * (4 * j + k):128 * (4 * j + k + 1)] = \
                WMh[128 * j:128 * (j + 1), 128 * k:128 * (k + 1)].T
    ident = np.eye(128, dtype=np.float32)
    memt = np.ascontiguousarray(inp["memory"].T)                  # [F, M] = [128, 512]
    membf = np.empty((128, 512), np.float32)   # memory row-chunks [m_local, F]
    for j in range(4):
        membf[:, 128 * j:128 * (j + 1)] = inp["memory"][128 * j:128 * (j + 1), :]
    wfct = np.empty((128, 512), np.float32)    # 0.5*W_fc.T chunks [128hk, O]
    for k in range(4):
        wfct[:, 128 * k:128 * (k + 1)] = 0.5 * inp["W_fc"].T[128 * k:128 * (k + 1), :]
    wqz = np.ascontiguousarray(Wq[:, 0:128].T)
    wqzp = np.ascontiguousarray(Wq[:, 128:256].T)

    scal = np.zeros((128, 8), np.float32)
    scal[:, 0], scal[:, 1] = dgz, inp["b_gz"]
    scal[:, 2], scal[:, 3] = dgzp, inp["b_gzp"]
    scal[:, 4], scal[:, 5] = b_q_eff, inp["b_fc"]

    import ml_dtypes
    wg = wg.astype(np.float16)
    selm = selm.astype(np.float16)
    biast16 = biast16.astype(np.float16)
    wm = wm.astype(np.float16)
    ident = ident.astype(np.float16)
    memt = memt.astype(np.float16)
    membf = membf.astype(ml_dtypes.bfloat16)
    shared = dict(wg=wg, selm=selm, biast16=biast16, wm=wm, ident=ident,
                  memt=memt, membf=membf, wfct=wfct, wqz=wqz, wqzp=wqzp, scal=scal)

    xm_rep = np.ascontiguousarray(
        np.repeat(inp["X_mean"][:T].T[:, :, None], BB, axis=2).reshape(F, TB))
    in_maps = []
    ch_names = ["x", "xl", "mask", "delta", "xlb", "dltb"]
    ch_idx = [0, 1, 2, 3, 4, 5]
    for core in range(NC):
        b0 = core * BB
        m = dict(shared)
        sl = inp["input"][b0:b0 + BB]          # [BB, 6, 100, F]
        for nm, ci in zip(ch_names, ch_idx):
            # [F, T, BB] -> [F, T*BB]
            m[nm] = np.ascontiguousarray(
                np.transpose(sl[:, ci, :T], (2, 1, 0)).reshape(F, TB))
        m["xmb"] = xm_rep
        in_maps.append(m)
    return in_maps


def kernel(**inputs):
    global _built
    from concourse import bass_utils
    if _built is None:
        _built = _build()
    in_maps = _prep_host(inputs)
    res = bass_utils.run_bass_kernel_spmd(_built, in_maps, core_ids=list(range(NC)))
    out = np.empty((B, 1, O), np.float32)
    for core in range(NC):
        out[core * BB:(core + 1) * BB, 0, :] = res.results[core]["o"].T
    return out


# revision 17
# speedup vs baseline: 1.0745x; 1.0745x over previous
"""Trainium2 Bass kernel for nn_LGnet (LSTM + memory attention recurrence).

Sharding: data-parallel over batch, B=256 -> 32 rows per core across 8 cores.
All on-chip state is kept transposed ([feature partitions, batch free]).

Algebraic restructure: logits = memory @ (ls_z[t] + WQ3F @ h)
                              = lgz[t] + WM @ h
where lgz = memory @ ls_z is precomputed on-chip (fp16, off the critical
path) and WM = 0.5 * memory @ WQ3F is a host-precomputed fp16 constant
(consuming h2 = 2h). This removes the per-step ls matmul + PSUM->SBUF
round-trip from the serial chain entirely: the logits matmuls depend only
on h and constants, so they issue the moment h is ready.

Step structure (per t):
  logits = lgz[t] + WM @ h2      (4x[ident + 4 wm] fp16 matmuls)
  e = exp(logits); s = ones-matmul sum; r = 1/s (bcast via ones matmul)
  gd = (e @ memory) * r          (bf16 matmuls)
  gates = bias + W_hh' @ h2 + W_ih' @ gdn  (fp16; bias via one K=16
          selector matmul - single closed PSUM group, rest is RMW;
          i/f/o rows pre-scaled 0.5, W_hh cols 0.5)
  Y = tanh(gates)                (ONE [128,512] activation)
  u = (Yf+1)*ch; v = (Yi+1)*Yg; cn2 = 2u+v  (fused scalar_tensor_tensor)
  TC = tanh(0.5*cn2); hb = (Yo+1)*TC -> fp16 (=2h); ch = 0.25*cn2 (=c/2)
The h-dependent gate matmuls fill the softmax window on the tensor engine.
"""
import os
import numpy as np
from contextlib import ExitStack

B, T, F, H, O, M = 256, 100, 128, 512, 128, 512
T = int(os.environ.get("LG_T", str(T)))   # debug override; harness uses 100
NC = 8
BB = B // NC          # 32 batch rows per core
TB = T * BB           # 3200 columns in (t, b) packing
NTCH = 4              # precompute T-chunks
TCH = T // NTCH       # 25 steps per chunk
CCH = TCH * BB        # 800 cols per chunk

_built = None


def _build():
    import concourse.bass as bass
    import concourse.tile as tile
    from concourse import bacc, mybir, bass_isa

    f32 = mybir.dt.float32
    bf16 = mybir.dt.bfloat16
    f16 = mybir.dt.float16
    AF = mybir.ActivationFunctionType
    ALU = mybir.AluOpType
    nc = bacc.Bacc("TRN2", target_bir_lowering=False, debug=False, num_devices=NC)
    # ---- DRAM tensors (per-core data fed via in_maps) ----
    dt_in = {}
    for name in ["x", "xl", "mask", "delta", "xlb", "dltb", "xmb"]:
        dt_in[name] = nc.dram_tensor(name, [F, TB], f32, kind="ExternalInput").ap()
    wg_d = nc.dram_tensor("wg", [128, 80 * 128], f16, kind="ExternalInput").ap()
    selm_d = nc.dram_tensor("selm", [16, 512], f16, kind="ExternalInput").ap()
    biast16_d = nc.dram_tensor("biast16", [16, 128], f16, kind="ExternalInput").ap()
    wm_d = nc.dram_tensor("wm", [128, 16 * 128], f16, kind="ExternalInput").ap()
    ident_d = nc.dram_tensor("ident", [128, 128], f16, kind="ExternalInput").ap()
    memt_d = nc.dram_tensor("memt", [128, 512], f16, kind="ExternalInput").ap()
    membf_d = nc.dram_tensor("membf", [128, 512], bf16, kind="ExternalInput").ap()
    wfct_d = nc.dram_tensor("wfct", [128, 512], f32, kind="ExternalInput").ap()
    wqz_d = nc.dram_tensor("wqz", [128, 128], f32, kind="ExternalInput").ap()
    wqzp_d = nc.dram_tensor("wqzp", [128, 128], f32, kind="ExternalInput").ap()
    scal_d = nc.dram_tensor("scal", [128, 8], f32, kind="ExternalInput").ap()
    # scal cols: 0 dgz, 1 bgz, 2 dgzp, 3 bgzp, 4 b_q_eff, 5 b_fc
    o_d = nc.dram_tensor("o", [O, BB], f32, kind="ExternalOutput").ap()
    dbg = os.environ.get("LG_DEBUG") == "1"
    if dbg:
        dbg_d = {nm: nc.dram_tensor(f"dbg_{nm}", shp, f32, kind="ExternalOutput").ap()
                 for nm, shp in [("eT", [128, 128]),
                                 ("gdn", [128, BB]), ("G", [128, 512]),
                                 ("Y", [128, 512]), ("cn2", [128, 128]),
                                 ("hb", [128, 128])]}

    with tile.TileContext(nc) as tc, ExitStack() as ctx:
        wpool = ctx.enter_context(tc.tile_pool(name="wpool", bufs=1))
        inp = ctx.enter_context(tc.tile_pool(name="inp", bufs=2))
        pre = ctx.enter_context(tc.tile_pool(name="pre", bufs=2))
        lszp = ctx.enter_context(tc.tile_pool(name="lszp", bufs=1))
        stp = ctx.enter_context(tc.tile_pool(name="stp", bufs=2))
        state = ctx.enter_context(tc.tile_pool(name="state", bufs=2))
        pers = ctx.enter_context(tc.tile_pool(name="pers", bufs=1))
        attn_ps = ctx.enter_context(tc.tile_pool(name="attn_ps", bufs=1, space="PSUM"))
        gates_ps = ctx.enter_context(tc.tile_pool(name="gates_ps", bufs=1, space="PSUM"))
        pre_ps = ctx.enter_context(tc.tile_pool(name="pre_ps", bufs=2, space="PSUM"))

        # ---- static weights into SBUF ----
        WG = wpool.tile([128, 80 * 128], f16, tag="WG")
        nc.sync.dma_start(WG[:], wg_d[:])
        SELM = wpool.tile([16, 512], f16, tag="SELM")
        nc.sync.dma_start(SELM[:], selm_d[:])
        BIAST16 = wpool.tile([16, 128], f16, tag="BIAST16")
        nc.sync.dma_start(BIAST16[:], biast16_d[:])
        WM = wpool.tile([128, 16 * 128], f16, tag="WM")
        nc.sync.dma_start(WM[:], wm_d[:])
        IDENT = wpool.tile([128, 128], f16, tag="IDENT")
        nc.sync.dma_start(IDENT[:], ident_d[:])
        MEMT = wpool.tile([128, 512], f16, tag="MEMT")
        nc.sync.dma_start(MEMT[:], memt_d[:])
        MEMBF = wpool.tile([128, 512], bf16, tag="MEMBF")
        nc.sync.dma_start(MEMBF[:], membf_d[:])
        WFCT = wpool.tile([128, 512], f32, tag="WFCT")
        nc.sync.dma_start(WFCT[:], wfct_d[:])
        WQZ = wpool.tile([128, 128], f32, tag="WQZ")
        nc.sync.dma_start(WQZ[:], wqz_d[:])
        WQZP = wpool.tile([128, 128], f32, tag="WQZP")
        nc.sync.dma_start(WQZP[:], wqzp_d[:])
        SCAL = wpool.tile([128, 8], f32, tag="SCAL")
        nc.sync.dma_start(SCAL[:], scal_d[:])
        ONESF = wpool.tile([128, 128], bf16, tag="ONESF")
        nc.vector.memset(ONESF[:], 1.0)
        ONESC = wpool.tile([128, 1], bf16, tag="ONESC")
        nc.vector.memset(ONESC[:], 1.0)

        dgz, bgz = SCAL[:, 0:1], SCAL[:, 1:2]
        dgzp, bgzp = SCAL[:, 2:3], SCAL[:, 3:4]
        bq_ap, bfc_ap = SCAL[:, 4:5], SCAL[:, 5:6]

        # ---- persistent tiles ----
        ls_z = lszp.tile([128, TB], f16, tag="ls_z")
        lgz = lszp.tile([128, 4 * TB], f16, tag="lgz")   # memory @ ls_z, 4 m-chunks

        hb = pers.tile([128, 128], f16, tag="hb")      # 2h, fp16
        ch = pers.tile([128, 128], f32, tag="ch")      # c/2, fp32
        nc.vector.memset(hb[:], 0.0)
        nc.vector.memset(ch[:], 0.0)

        # ---- precompute z/zp, ls_z and lgz = memory @ ls_z in T-chunks ----
        with nc.named_scope("precompute"):
            for cc in range(NTCH):
                sl = slice(cc * CCH, (cc + 1) * CCH)
                chd = {}
                for name in ["x", "xl", "mask", "delta", "xlb", "dltb", "xmb"]:
                    t_ = inp.tile([128, CCH], f32, tag=f"in_{name}")
                    nc.sync.dma_start(t_[:], dt_in[name][:, sl])
                    chd[name] = t_

                def zchain(dsrc, xlsrc, dg, bg, tag):
                    r1 = pre.tile([128, CCH], f32, tag="tA")
                    nc.scalar.activation(r1[:], dsrc[:], AF.Relu, scale=dg, bias=bg)
                    dz = pre.tile([128, CCH], f32, tag="tB")
                    nc.scalar.activation(dz[:], r1[:], AF.Exp, scale=-1.0)
                    u = pre.tile([128, CCH], f32, tag="tA")
                    nc.vector.tensor_tensor(u[:], xlsrc[:], chd["xmb"][:], ALU.subtract)
                    v = pre.tile([128, CCH], f32, tag="tB2")
                    nc.vector.tensor_tensor(v[:], dz[:], u[:], ALU.mult)
                    w = pre.tile([128, CCH], f32, tag="tC")
                    nc.vector.tensor_tensor(w[:], v[:], chd["xmb"][:], ALU.add)
                    d_ = pre.tile([128, CCH], f32, tag="tA")
                    nc.vector.tensor_tensor(d_[:], chd["x"][:], w[:], ALU.subtract)
                    e2 = pre.tile([128, CCH], f32, tag="tB")
                    nc.vector.tensor_tensor(e2[:], chd["mask"][:], d_[:], ALU.mult)
                    z_ = pre.tile([128, CCH], f32, tag=f"z{tag}")
                    nc.vector.tensor_tensor(z_[:], w[:], e2[:], ALU.add)
                    return z_

                z_c = zchain(chd["delta"], chd["xl"], dgz, bgz, "z")
                zp_c = zchain(chd["dltb"], chd["xlb"], dgzp, bgzp, "p")

                for off in range(0, CCH, 512):
                    n = min(512, CCH - off)
                    pp = pre_ps.tile([128, 512], f32, tag="pp")
                    nc.tensor.matmul(pp[:, :n], lhsT=WQZ[:], rhs=z_c[:, off:off + n],
                                     start=True, stop=False)
                    nc.tensor.matmul(pp[:, :n], lhsT=WQZP[:], rhs=zp_c[:, off:off + n],
                                     start=False, stop=True)
                    nc.scalar.activation(ls_z[:, cc * CCH + off: cc * CCH + off + n],
                                         pp[:, :n], AF.Identity, bias=bq_ap)
                # lgz chunks for this cc: 4 m-chunks x (512+288) cols
                for j in range(4):
                    for off in range(0, CCH, 512):
                        n = min(512, CCH - off)
                        pq = pre_ps.tile([128, 512], f32, tag="pq")
                        nc.tensor.matmul(pq[:, :n], lhsT=MEMT[:, 128 * j:128 * (j + 1)],
                                         rhs=ls_z[:, cc * CCH + off: cc * CCH + off + n],
                                         start=True, stop=True)
                        dst = lgz[:, j * TB + cc * CCH + off: j * TB + cc * CCH + off + n]
                        nc.scalar.activation(dst, pq[:, :n], AF.Identity)

        # gate-group matmul helpers; gates PSUM is split into three tiles
        # (f / ig / o) because PSUM dependency tracking is bank-coarse -
        # separate banks let each tanh start when only its block is done.
        def g_tile(tiles, g):
            if g < 4:
                return tiles[0], 32 * g
            if g < 12:
                return tiles[1], 32 * (g - 4)
            return tiles[2], 32 * (g - 12)

        def g_h(tiles, g):
            pt, c0 = g_tile(tiles, g)
            for k in range(4):
                nc.tensor.matmul(pt[:, c0:c0 + 32],
                                 lhsT=WG[:, 128 * (g * 5 + 1 + k):128 * (g * 5 + 2 + k)],
                                 rhs=hb[:, 32 * k:32 * k + 32],
                                 start=False, stop=False, skip_group_check=True)

        def g_gd(tiles, g, gdn):
            pt, c0 = g_tile(tiles, g)
            nc.tensor.matmul(pt[:, c0:c0 + 32],
                             lhsT=WG[:, 128 * (g * 5):128 * (g * 5 + 1)],
                             rhs=gdn[:], start=False, stop=True, skip_group_check=True)

        # ---- recurrence ----
        for t in range(T):
            with nc.named_scope(f"step{t}" if t % 10 == 0 else "step"):
                pa = attn_ps.tile([128, 512], f32, tag="pa")
                pgf = gates_ps.tile([128, 128], f32, tag="pgf")
                pgig = gates_ps.tile([128, 256], f32, tag="pgig")
                pgo = gates_ps.tile([128, 128], f32, tag="pgo")
                pgt = (pgf, pgig, pgo)
                # logits^T chunk j = lgz[t] chunk j + sum_k WM(j,k) @ h2 chunk k
                for j in range(4):
                    rg = pa[:, 128 + 32 * j:128 + 32 * (j + 1)]
                    nc.tensor.matmul(rg, lhsT=IDENT[:],
                                     rhs=lgz[:, j * TB + 32 * t: j * TB + 32 * t + 32],
                                     start=True, stop=False)
                    for k in range(4):
                        nc.tensor.matmul(rg, lhsT=WM[:, 128 * (4 * j + k):128 * (4 * j + k + 1)],
                                         rhs=hb[:, 32 * k:32 * k + 32],
                                         start=False, stop=(k == 3))
                # bias via K=16 selector matmuls, one closed group per gate
                # block; everything after is RMW
                nc.tensor.matmul(pgf[:, 0:128], lhsT=BIAST16[:], rhs=SELM[:, 0:128],
                                 start=True, stop=True)
                nc.tensor.matmul(pgig[:, 0:256], lhsT=BIAST16[:], rhs=SELM[:, 128:384],
                                 start=True, stop=True)
                nc.tensor.matmul(pgo[:, 0:128], lhsT=BIAST16[:], rhs=SELM[:, 384:512],
                                 start=True, stop=True)
                eT = stp.tile([128, 128], bf16, tag="eT")
                nc.scalar.activation(eT[:], pa[:, 128:256], AF.Exp)
                # fold the 4 m-chunks on vector, then all-reduce the [128,32]
                # partials over partitions on gpsimd (result broadcast to all
                # partitions -> no ones-matmul, no broadcast matmul)
                eF = stp.tile([128, BB], f32, tag="eF")
                nc.vector.tensor_reduce(eF[:], eT[:].rearrange("p (c b) -> p b c", c=4),
                                        axis=mybir.AxisListType.X, op=ALU.add)
                eAR = stp.tile([128, BB], f32, tag="eAR")
                nc.gpsimd.partition_all_reduce(eAR[:], eF[:], channels=128,
                                               reduce_op=bass_isa.ReduceOp.add)
                # gd^T = memory.T-chunks @ e^T  (bf16)
                for j in range(4):
                    nc.tensor.matmul(pa[:, 256:288], lhsT=MEMBF[:, 128 * j:128 * (j + 1)],
                                     rhs=eT[:, 32 * j:32 * j + 32],
                                     start=(j == 0), stop=(j == 3))
                # gates: h part groups 0..9 (fills the softmax window)
                for g in range(0, 10):
                    g_h(pgt, g)
                R = stp.tile([128, BB], f32, tag="R")
                nc.vector.reciprocal(R[:], eAR[:])
                for g in range(10, 16):
                    g_h(pgt, g)
                gdn = stp.tile([128, BB], f16, tag="gdn")
                nc.vector.tensor_tensor(gdn[:], pa[:, 256:288], R[:], ALU.mult)
                # gd matmuls and tanh interleaved per gate block (f, then ig,
                # then o) so each activation starts as soon as its block is done
                Y = stp.tile([128, 512], f32, tag="Y")
                for g in range(0, 4):
                    g_gd(pgt, g, gdn)
                nc.scalar.activation(Y[:, 0:128], pgf[:], AF.Tanh)
                for g in range(4, 12):
                    g_gd(pgt, g, gdn)
                nc.scalar.activation(Y[:, 128:384], pgig[:], AF.Tanh)
                for g in range(12, 16):
                    g_gd(pgt, g, gdn)
                nc.scalar.activation(Y[:, 384:512], pgo[:], AF.Tanh)
                # pointwise (group order f,i,g,o): u=(Yf+1)*ch  v=(Yi+1)*Yg
                u = stp.tile([128, 128], f32, tag="u")
                nc.vector.scalar_tensor_tensor(u[:], Y[:, 0:128], 1.0, ch[:],
                                               ALU.add, ALU.mult)
                v = stp.tile([128, 128], f32, tag="v")
                nc.vector.scalar_tensor_tensor(v[:], Y[:, 128:256], 1.0, Y[:, 256:384],
                                               ALU.add, ALU.mult)
                cn2 = state.tile([128, 128], f32, tag="cn2")
                nc.vector.scalar_tensor_tensor(cn2[:], v[:], 0.5, u[:],
                                               ALU.mult, ALU.add)
                TC = stp.tile([128, 128], f32, tag="TC")
                nc.scalar.activation(TC[:], cn2[:], AF.Tanh)
                hb_new = state.tile([128, 128], f16, tag="hb2")
                nc.vector.scalar_tensor_tensor(hb_new[:], Y[:, 384:512], 1.0, TC[:],
                                               ALU.add, ALU.mult)
                ch_new = state.tile([128, 128], f32, tag="ch2")
                nc.scalar.mul(ch_new[:], cn2[:], 0.5)
                if dbg and t == int(os.environ.get("LG_DBGT", "0")):
                    eTf = stp.tile([128, 128], f32, tag="eTf")
                    nc.vector.tensor_copy(eTf[:], eT[:])
                    nc.sync.dma_start(dbg_d["eT"][:], eTf[:])
                    gdnf = stp.tile([128, BB], f32, tag="gdnf")
                    nc.vector.tensor_copy(gdnf[:], gdn[:])
                    nc.sync.dma_start(dbg_d["gdn"][:], gdnf[:])
                    Gd = stp.tile([128, 512], f32, tag="Gd")
                    nc.scalar.activation(Gd[:, 0:128], pgf[:], AF.Identity)
                    nc.scalar.activation(Gd[:, 128:384], pgig[:], AF.Identity)
                    nc.scalar.activation(Gd[:, 384:512], pgo[:], AF.Identity)
                    nc.sync.dma_start(dbg_d["G"][:], Gd[:])
                    nc.sync.dma_start(dbg_d["Y"][:], Y[:])
                    nc.sync.dma_start(dbg_d["cn2"][:], cn2[:])
                    hbf = stp.tile([128, 128], f32, tag="hbf")
                    nc.vector.tensor_copy(hbf[:], hb_new[:])
                    nc.sync.dma_start(dbg_d["hb"][:], hbf[:])
                if t == T - 1:
                    h2f = stp.tile([128, 128], f32, tag="h2f")
                    nc.vector.scalar_tensor_tensor(h2f[:], Y[:, 384:512], 1.0, TC[:],
                                                   ALU.add, ALU.mult)
                hb, ch = hb_new, ch_new

        # ---- final output: out^T = 0.5*W_fc @ h2 + b_fc ----
        with nc.named_scope("final"):
            pf = attn_ps.tile([128, 512], f32, tag="pa")
            for k in range(4):
                nc.tensor.matmul(pf[:, 0:32], lhsT=WFCT[:, 128 * k:128 * (k + 1)],
                                 rhs=h2f[:, 32 * k:32 * k + 32],
                                 start=(k == 0), stop=(k == 3))
            outt = stp.tile([O, BB], f32, tag="outt")
            nc.scalar.activation(outt[:], pf[:, 0:32], AF.Identity, bias=bfc_ap)
            nc.sync.dma_start(o_d[:], outt[:])

    nc.compile()
    return nc


def _prep_host(inputs):
    """Host-side: fold weights, build per-core input maps."""
    inp = {k: np.asarray(v, np.float32) for k, v in inputs.items()}
    dgz = np.ascontiguousarray(np.diag(inp["W_gz"]))
    dgzp = np.ascontiguousarray(np.diag(inp["W_gzp"]))
    Wq = inp["W_q"]
    WQ3F = (Wq[:, 2 * F:] @ inp["W_fc"]).astype(np.float32)       # [F, H]
    b_q_eff = (inp["b_q"] + Wq[:, 2 * F:] @ inp["b_fc"]).astype(np.float32)
    bias_g = (inp["b_ih"] + inp["b_hh"]).astype(np.float32)       # [2048]

    # gates weights: Wcat.T tiles; tile (g, k): k=0 -> W_ih cols, k=1..4 -> W_hh
    # pre-scale: i/f/o gate columns (g//4 != 2) x0.5 for the tanh-sigmoid trick,
    # W_hh part (k>=1) x0.5 because h state is stored as 2h.
    WcatT = np.concatenate([inp["W_ih"], inp["W_hh"]], axis=1).T  # [640, 2048]
    # group order in pg columns: [f(4..7), i(0..3), g(8..11), o(12..15)] so a
    # small tanh over the f block can unblock the c-update early
    GORD = [4, 5, 6, 7, 0, 1, 2, 3, 8, 9, 10, 11, 12, 13, 14, 15]
    wg = np.empty((128, 80 * 128), np.float32)
    for gn, g in enumerate(GORD):
        gate_sc = 0.5 if g // 4 != 2 else 1.0
        for k in range(5):
            sc = gate_sc * (0.5 if k >= 1 else 1.0)
            wg[:, 128 * (gn * 5 + k):128 * (gn * 5 + k + 1)] = \
                sc * WcatT[128 * k:128 * (k + 1), 128 * g:128 * (g + 1)]

    selm = np.zeros((16, 512), np.float32)
    for g in range(16):
        selm[g, 32 * g:32 * (g + 1)] = 1.0
    biast16 = np.empty((16, 128), np.float32)
    for gn, g in enumerate(GORD):
        gate_sc = 0.5 if g // 4 != 2 else 1.0
        biast16[gn, :] = gate_sc * bias_g[128 * g:128 * (g + 1)]

    # WM = 0.5 * memory @ WQ3F  [M, H]; tile (j, k) = WM[mj, hk].T
    WMh = 0.5 * (inp["memory"] @ WQ3F)                            # [512, 512]
    wm = np.empty((128, 16 * 128), np.float32)
    for j in range(4):
        for k in range(4):
            wm[:, 128 * (4 * j + k):128 * (4 * j + k + 1)] = \
                WMh[128 * j:128 * (j + 1), 128 * k:128 * (k + 1)].T
    ident = np.eye(128, dtype=np.float32)
    memt = np.ascontiguousarray(inp["memory"].T)                  # [F, M] = [128, 512]
    membf = np.empty((128, 512), np.float32)   # memory row-chunks [m_local, F]
    for j in range(4):
        membf[:, 128 * j:128 * (j + 1)] = inp["memory"][128 * j:128 * (j + 1), :]
    wfct = np.empty((128, 512), np.float32)    # 0.5*W_fc.T chunks [128hk, O]
    for k in range(4):
        wfct[:, 128 * k:128 * (k + 1)] = 0.5 * inp["W_fc"].T[128 * k:128 * (k + 1), :]
    wqz = np.ascontiguousarray(Wq[:, 0:128].T)
    wqzp = np.ascontiguousarray(Wq[:, 128:256].T)

    scal = np.zeros((128, 8), np.float32)
    scal[:, 0], scal[:, 1] = dgz, inp["b_gz"]
    scal[:, 2], scal[:, 3] = dgzp, inp["b_gzp"]
    scal[:, 4], scal[:, 5] = b_q_eff, inp["b_fc"]

    import ml_dtypes
    wg = wg.astype(np.float16)
    selm = selm.astype(np.float16)
    biast16 = biast16.astype(np.float16)
    wm = wm.astype(np.float16)
    ident = ident.astype(np.float16)
    memt = memt.astype(np.float16)
    membf = membf.astype(ml_dtypes.bfloat16)
    shared = dict(wg=wg, selm=selm, biast16=biast16, wm=wm, ident=ident,
                  memt=memt, membf=membf, wfct=wfct, wqz=wqz, wqzp=wqzp, scal=scal)

    xm_rep = np.ascontiguousarray(
        np.repeat(inp["X_mean"][:T].T[:, :, None], BB, axis=2).reshape(F, TB))
    in_maps = []
    ch_names = ["x", "xl", "mask", "delta", "xlb", "dltb"]
    ch_idx = [0, 1, 2, 3, 4, 5]
    for core in range(NC):
        b0 = core * BB
        m = dict(shared)
        sl = inp["input"][b0:b0 + BB]          # [BB, 6, 100, F]
        for nm, ci in zip(ch_names, ch_idx):
            # [F, T, BB] -> [F, T*BB]
            m[nm] = np.ascontiguousarray(
                np.transpose(sl[:, ci, :T], (2, 1, 0)).reshape(F, TB))
        m["xmb"] = xm_rep
        in_maps.append(m)
    return in_maps


def kernel(**inputs):
    global _built
    from concourse import bass_utils
    if _built is None:
        _built = _build()
    in_maps = _prep_host(inputs)
    res = bass_utils.run_bass_kernel_spmd(_built, in_maps, core_ids=list(range(NC)))
    out = np.empty((B, 1, O), np.float32)
    for core in range(NC):
        out[core * BB:(core + 1) * BB, 0, :] = res.results[core]["o"].T
    return out
